# revision 17
# baseline (speedup 1.0000x reference)
"""GAT (2-layer, 8-head) Bass kernel for one TRN2 chip (8 NeuronCores).

Row-sharded: core c owns rows [c*L, (c+1)*L) of the N=4096 nodes.
All big on-chip tensors live in a TRANSPOSED layout [j (partitions), i (free)]
so that the softmax row-sum and att@Wh both run on the TensorEngine
(contraction over j = partition dim), and the attention logits
e[i,j] = leaky_relu(src[i]+dst[j]) are built by rank-2 outer-product matmuls.

Per layer: local Wh^T = W_aug^T x^T on PE (W_aug has W@a1, W@a2 folded in as
extra columns, so src/dst come out of the same matmul), AllGather of Wh + dst,
then per 4-j-tile chunk: PE outer-add -> ACT Lrelu -> ACT Exp -> DVE mask-mult
-> PE att@Wh(+rowsum via ones column). ELU / log_softmax tails on small
[H, 512] tensors. Output returned transposed [40, 512] per core; host
re-transposes.
"""

import sys

sys.path.insert(0, "/opt/trn_rl_repo")

import numpy as np
import ml_dtypes

from concourse import bass, bacc, mybir, tile
from concourse.bass_utils import run_bass_kernel_spmd

BF16 = mybir.dt.bfloat16
F32 = mybir.dt.float32
AF = mybir.ActivationFunctionType
ALU = mybir.AluOpType

NP_BF16 = ml_dtypes.bfloat16

# Problem constants (full size)
N = 4096
NCORES = 8
NFEAT = 512
NHID = 64
NCLASS = 40
NHEADS = 8
ALPHA = 0.2


def build_gat_nc(
    n=N,
    ncores=NCORES,
    nfeat=NFEAT,
    nhid=NHID,
    nclass=NCLASS,
    nheads=NHEADS,
    chunk_jt=4,
):
    """Build the SPMD Bass program (same program on every core)."""
    local = n // ncores  # rows per core
    njt = n // 128  # j tiles of 128 partitions
    jt_per_core = local // 128
    nft = nfeat // 128  # feature tiles (contraction dim of layer-1 matmul)
    assert local % 128 == 0 and n % 128 == 0 and nfeat % 128 == 0
    assert njt % chunk_jt == 0
    nchunks = njt // chunk_jt
    cw = chunk_jt * local  # chunk free-size in fp32 elements (psum)
    assert chunk_jt * 128 * 4 <= 8 * 2048  # chunk fits PSUM comfortably
    # W_aug padded to 128 cols so src/dst land on legal partition bases:
    # cols [0:nhid]=W, col 64=W@a1 (src), col 96=W@a2 (dst), zeros elsewhere
    w1c = 128
    w2c = 128

    nc = bacc.Bacc(
        "TRN2", target_bir_lowering=False, debug=False, num_devices=ncores
    )

    # ---------------- I/O ----------------
    xT_d = nc.dram_tensor("xT", [nfeat, local], BF16, kind="ExternalInput")
    wall_d = nc.dram_tensor(
        "wall", [nfeat, nheads * w1c], BF16, kind="ExternalInput"
    )
    # rows 0..nfeat-1 = [W_out | W_out@a1 | W_out@a2]; last row = column sums
    wout_d = nc.dram_tensor("wout", [nfeat + 1, w2c], BF16, kind="ExternalInput")
    maskT_d = nc.dram_tensor("maskT", [n, local], BF16, kind="ExternalInput")
    ones_d = nc.dram_tensor("onesrow", [1, n], BF16, kind="ExternalInput")
    out_d = nc.dram_tensor("outT", [nclass, local], F32, kind="ExternalOutput")

    with tile.TileContext(nc) as tc:
        with (
            tc.tile_pool(name="const", bufs=1) as const,
            tc.tile_pool(name="stage", bufs=2) as stage,
            tc.tile_pool(name="upool", bufs=2) as upool,
            tc.tile_pool(name="ppool", bufs=2) as ppool,
            tc.tile_pool(name="pmpool", bufs=2) as pmpool,
            tc.tile_pool(name="small", bufs=2) as small,
            tc.tile_pool(name="spsum", bufs=1, space="PSUM") as spsum,
            tc.tile_pool(name="attpsum", bufs=2, space="PSUM") as attpsum,
            tc.tile_pool(name="mpsum", bufs=2, space="PSUM") as mpsum,
            tc.tile_pool(name="dram", bufs=1, space="DRAM") as dram,
        ):
            # ---------------- static SBUF tensors ----------------
            xt = []
            wl = []
            for q in range(nft):
                xtq = const.tile([128, local], BF16, name=f"xt{q}", tag=f"xt{q}")
                nc.sync.dma_start(xtq[:], xT_d[q * 128 : (q + 1) * 128, :])
                xt.append(xtq)
                wlq = const.tile(
                    [128, nheads * w1c], BF16, name=f"wl{q}", tag=f"wl{q}"
                )
                nc.sync.dma_start(wlq[:], wall_d[q * 128 : (q + 1) * 128, :])
                wl.append(wlq)
            wo = []
            for q in range(nft):
                woq = const.tile([128, w2c], BF16, name=f"wo{q}", tag=f"wo{q}")
                nc.sync.dma_start(woq[:], wout_d[q * 128 : (q + 1) * 128, :])
                wo.append(woq)
            wcorr = const.tile([1, w2c], BF16, name="wcorr", tag="wcorr")
            nc.sync.dma_start(wcorr[:], wout_d[nfeat : nfeat + 1, :])

            # mask, [128, njt, local] bf16 (transposed adjacency, {0,1})
            mask_sb = const.tile([128, njt, local], BF16, name="mask_sb", tag="mask")
            mt_r = maskT_d.ap().rearrange("(jt p) i -> p jt i", p=128)
            ndma = 4
            per = njt // ndma
            for q in range(ndma):
                nc.sync.dma_start(
                    mask_sb[:, q * per : (q + 1) * per, :],
                    mt_r[:, q * per : (q + 1) * per, :],
                )

            ones_col = const.tile([128, 1], BF16, name="ones_col", tag="ones_col")
            nc.vector.memset(ones_col[:], 1.0)
            ones_row = const.tile([1, 128], F32, name="ones_row", tag="ones_row")
            nc.vector.memset(ones_row[:], 1.0)
            negones = const.tile([1, local], BF16, name="negones", tag="negones")
            nc.vector.memset(negones[:], -1.0)

            # outer-product operand tables. matmul operands must start at
            # partition 0/32/64, so pack 3 (dst,ones)/(ones,src) pairs per
            # 66-row tile at bases 0/32/64. Slot k (0..8): tile k//3, base
            # 32*(k%3). Slots 0..7 = heads, slot 8 = layer 2.
            do2t = []
            so2t = []
            for t in range(3):
                d = const.tile([66, n], BF16, name=f"do2_{t}", tag=f"do2_{t}")
                s = const.tile([66, local], BF16, name=f"so2_{t}", tag=f"so2_{t}")
                do2t.append(d)
                so2t.append(s)
            def pair(k):
                return do2t[k // 3], so2t[k // 3], 32 * (k % 3)
            for k in range(nheads + 1):
                d, s, b = pair(k)
                nc.sync.dma_start(d[b + 1 : b + 2, :], ones_d[:, :])
                nc.vector.memset(s[b : b + 1, :], 1.0)

            # gathered Wh in lhsT form + ones column: [128, njt, nhid+1]
            whaug = []
            for h in range(nheads):
                wa = const.tile(
                    [128, njt, nhid + 1], BF16, name=f"whaug{h}", tag=f"whaug{h}"
                )
                nc.vector.memset(wa[:, :, nhid : nhid + 1], 1.0)
                whaug.append(wa)
            whaug2 = const.tile([128, njt, 65], BF16, name="whaug2", tag="whaug2")
            nc.vector.memset(whaug2[:, :, nclass:64], 0.0)
            nc.vector.memset(whaug2[:, :, 64:65], 1.0)

            # h~ = elu(layer1 out)+1, transposed: [nfeat=nheads*nhid, local]
            ht = []
            for q in range(nft):
                htq = const.tile([128, local], BF16, name=f"ht{q}", tag=f"ht{q}")
                ht.append(htq)

            # ---------------- DRAM bounce buffers ----------------
            wh_pay = dram.tile([nheads * nhid, local], BF16, name="wh_pay")
            sd_pay = dram.tile([nheads, local], BF16, name="sd_pay")
            wh_gath = dram.tile(
                [ncores * nheads * nhid, local],
                BF16,
                name="wh_gath",
                addr_space="Shared",
            )
            sd_gath = dram.tile(
                [ncores * nheads, local], BF16, name="sd_gath", addr_space="Shared"
            )
            wh2_pay = dram.tile([nclass, local], BF16, name="wh2_pay")
            sd2_pay = dram.tile([1, local], BF16, name="sd2_pay")
            wh2_gath = dram.tile(
                [ncores * nclass, local], BF16, name="wh2_gath", addr_space="Shared"
            )
            sd2_gath = dram.tile(
                [ncores, local], BF16, name="sd2_gath", addr_space="Shared"
            )

            # ---------------- phase 1: local Wh^T per head ----------------
            for h in range(nheads):
                whT_ps = mpsum.tile([w1c, local], F32, name=f"whT{h}", tag="whT")
                for q in range(nft):
                    nc.tensor.matmul(
                        whT_ps[:],
                        wl[q][:, h * w1c : (h + 1) * w1c],
                        xt[q][:],
                        start=(q == 0),
                        stop=(q == nft - 1),
                    )
                # payload rows: Wh^T (bf16)
                whT_sb = stage.tile([nhid, local], BF16, name="whT_sb", tag="whT_sb")
                nc.scalar.copy(whT_sb[:], whT_ps[0:nhid, :])
                nc.sync.dma_start(
                    wh_pay[h * nhid : (h + 1) * nhid, :], whT_sb[:]
                )
                # src row (64) -> so2 (via SBUF-to-SBUF DMA; odd partition
                # writes are not legal for compute engines), dst row (96) ->
                # sd payload
                sh, ssb, sb_ = pair(h)
                src_sb = stage.tile([1, local], BF16, name="src_sb", tag="src_sb")
                nc.scalar.copy(src_sb[:], whT_ps[64:65, :])
                nc.sync.dma_start(ssb[sb_ + 1 : sb_ + 2, :], src_sb[:])
                dst_sb = stage.tile([1, local], BF16, name="dst_sb", tag="dst_sb")
                nc.scalar.copy(dst_sb[:], whT_ps[96:97, :])
                nc.sync.dma_start(sd_pay[h : h + 1, :], dst_sb[:])

            # ---------------- phase 2: AllGather ----------------
            grp = [list(range(ncores))]
            nc.gpsimd.collective_compute(
                "AllGather",
                ALU.bypass,
                replica_groups=grp,
                ins=[wh_pay[:].opt()],
                outs=[wh_gath[:].opt()],
            )
            nc.gpsimd.collective_compute(
                "AllGather",
                ALU.bypass,
                replica_groups=grp,
                ins=[sd_pay[:].opt()],
                outs=[sd_gath[:].opt()],
            )

            # ---------------- phase 3: build whaug, do2 ----------------
            # wh_gath row = c*(H*nhid) + h*nhid + d, col = il
            # whaug_h[p, jt, d] = Wh_h[jt*128+p, d];  c = jt // jt_per_core,
            # il = (jt % jt_per_core)*128 + p
            wg = wh_gath[:].rearrange(
                "(c h d) (jtl p) -> c h jtl p d", c=ncores, h=nheads, p=128
            )
            for h in range(nheads):
                for jt in range(njt):
                    c, jtl = divmod(jt, jt_per_core)
                    nc.sync.dma_start(
                        whaug[h][:, jt, 0:nhid],
                        wg[c, h, jtl],
                    )
            # sd_gath row = c*H + h -> do2 slot-row for head h, [1, ncores, local]
            sg = sd_gath[:].rearrange("(c h) i -> h c i", h=nheads)
            for h in range(nheads):
                d, _, b = pair(h)
                d_r = d[b : b + 1, :].rearrange("p (c i) -> p c i", c=ncores)
                nc.sync.dma_start(d_r, sg[h : h + 1])

            # ---------------- phase 4: layer-1 attention main loop ----------
            def att_layer(nh, whaug_h, slot):
                """One head's attention: att psum tile [65, local], rowsum row 64."""
                do2_t, so2_t, b = pair(slot)
                att_ps = attpsum.tile([65, local], F32, name="att_ps", tag="att")
                for ch in range(nchunks):
                    s_ps = spsum.tile([128, cw], F32, name="s_ps", tag="s_ps")
                    for q in range(chunk_jt):
                        jt = ch * chunk_jt + q
                        nc.tensor.matmul(
                            s_ps[:, q * local : (q + 1) * local],
                            do2_t[b : b + 2, jt * 128 : (jt + 1) * 128],
                            so2_t[b : b + 2, :],
                            start=True,
                            stop=True,
                        )
                    u = upool.tile([128, cw], F32, name="u", tag="u")
                    nc.scalar.activation(u[:], s_ps[:], AF.Lrelu, alpha=ALPHA)
                    p = ppool.tile([128, cw], BF16, name="p", tag="p")
                    nc.scalar.activation(p[:], u[:], AF.Exp)
                    pm = pmpool.tile([128, cw], BF16, name="pm", tag="pm")
                    pv = p[:].rearrange("p (q i) -> p q i", q=chunk_jt)
                    pmv = pm[:].rearrange("p (q i) -> p q i", q=chunk_jt)
                    nc.vector.tensor_tensor(
                        pmv,
                        pv,
                        mask_sb[:, ch * chunk_jt : (ch + 1) * chunk_jt, :],
                        ALU.mult,
                    )
                    for q in range(chunk_jt):
                        jt = ch * chunk_jt + q
                        nc.tensor.matmul(
                            att_ps[:],
                            whaug_h[:, jt, :],
                            pm[:, q * local : (q + 1) * local],
                            start=(jt == 0),
                            stop=(jt == njt - 1),
                        )
                return att_ps

            def elu_plus1(z_sb, out_view):
                """out = elu(z)+1 = (z>=0 ? 1+z : exp(z)), elementwise."""
                pp = z_sb.shape[0]
                m1 = small.tile([pp, local], F32, name="m1", tag="elu_m1")
                nc.vector.tensor_scalar(m1[:], z_sb[:], 0.0, None, ALU.min)
                e1 = small.tile([pp, local], F32, name="e1", tag="elu_e1")
                nc.scalar.activation(e1[:], m1[:], AF.Exp)
                r1 = small.tile([pp, local], F32, name="r1", tag="elu_r1")
                nc.vector.tensor_scalar(r1[:], z_sb[:], 0.0, None, ALU.max)
                nc.vector.tensor_tensor(out_view, e1[:], r1[:], ALU.add)

            for h in range(nheads):
                att_ps = att_layer(nhid, whaug[h], h)
                # normalize + elu + 1 -> ht
                rc = small.tile([1, local], F32, name="rc", tag="rc")
                nc.vector.reciprocal(rc[:], att_ps[64:65, :])
                bc_ps = mpsum.tile([nhid, local], F32, name="bc_ps", tag="whT")
                nc.tensor.matmul(
                    bc_ps[:], ones_row[:, 0:nhid], rc[:], start=True, stop=True
                )
                bc_sb = small.tile([nhid, local], F32, name="bc_sb", tag="bc_sb")
                nc.scalar.copy(bc_sb[:], bc_ps[:])
                z_sb = small.tile([nhid, local], F32, name="z_sb", tag="z_sb")
                nc.vector.tensor_tensor(z_sb[:], att_ps[0:nhid, :], bc_sb[:], ALU.mult)
                q, r = divmod(h * nhid, 128)
                elu_plus1(z_sb, ht[q][r : r + nhid, :])

            # ---------------- phase 5: layer 2 ----------------
            wh2T_ps = mpsum.tile([w2c, local], F32, name="wh2T", tag="whT")
            for q in range(nft):
                nc.tensor.matmul(
                    wh2T_ps[:], wo[q][:], ht[q][:], start=(q == 0), stop=False
                )
            # subtract column-sums (the h~ = h+1 correction)
            nc.tensor.matmul(
                wh2T_ps[:], wcorr[:], negones[:], start=False, stop=True
            )
            wh2_sb = stage.tile([nclass, local], BF16, name="wh2_sb", tag="wh2_sb")
            nc.scalar.copy(wh2_sb[:], wh2T_ps[0:nclass, :])
            nc.sync.dma_start(wh2_pay[:], wh2_sb[:])
            d2t, s2t, b2 = pair(nheads)
            src2_sb = stage.tile([1, local], BF16, name="src2_sb", tag="src_sb")
            nc.scalar.copy(src2_sb[:], wh2T_ps[64:65, :])
            nc.sync.dma_start(s2t[b2 + 1 : b2 + 2, :], src2_sb[:])
            dst2_sb = stage.tile([1, local], BF16, name="dst2_sb", tag="dst_sb")
            nc.scalar.copy(dst2_sb[:], wh2T_ps[96:97, :])
            nc.sync.dma_start(sd2_pay[:], dst2_sb[:])

            nc.gpsimd.collective_compute(
                "AllGather",
                ALU.bypass,
                replica_groups=grp,
                ins=[wh2_pay[:].opt()],
                outs=[wh2_gath[:].opt()],
            )
            nc.gpsimd.collective_compute(
                "AllGather",
                ALU.bypass,
                replica_groups=grp,
                ins=[sd2_pay[:].opt()],
                outs=[sd2_gath[:].opt()],
            )

            wg2 = wh2_gath[:].rearrange(
                "(c d) (jtl p) -> c jtl p d", c=ncores, p=128
            )
            for jt in range(njt):
                c, jtl = divmod(jt, jt_per_core)
                nc.sync.dma_start(whaug2[:, jt, 0:nclass], wg2[c, jtl])
            do22_r = d2t[b2 : b2 + 1, :].rearrange("p (c i) -> p c i", c=ncores)
            nc.sync.dma_start(
                do22_r, sd2_gath[:].rearrange("(o c) i -> o c i", o=1)
            )

            att2_ps = att_layer(nclass, whaug2, nheads)

            # ---------------- phase 6: final elu + log_softmax ------------
            rc2 = small.tile([1, local], F32, name="rc2", tag="rc")
            nc.vector.reciprocal(rc2[:], att2_ps[64:65, :])
            bc2_ps = mpsum.tile([nclass, local], F32, name="bc2_ps", tag="whT")
            nc.tensor.matmul(
                bc2_ps[:], ones_row[:, 0:nclass], rc2[:], start=True, stop=True
            )
            bc2_sb = small.tile([nclass, local], F32, name="bc2_sb", tag="bc_sb")
            nc.scalar.copy(bc2_sb[:], bc2_ps[:])
            v_sb = small.tile([nclass, local], F32, name="v_sb", tag="z_sb")
            nc.vector.tensor_tensor(v_sb[:], att2_ps[0:nclass, :], bc2_sb[:], ALU.mult)
            vt = small.tile([nclass, local], F32, name="vt", tag="vt")
            elu_plus1(v_sb, vt[:])
            # log_softmax(v-1) = vt - ln(sum exp(vt))  (the -1 cancels)
            ev = small.tile([nclass, local], F32, name="ev", tag="ev")
            nc.scalar.activation(ev[:], vt[:], AF.Exp)
            ones_colf = const.tile([128, 1], F32, name="ones_colf", tag="ones_colf")
            nc.vector.memset(ones_colf[:], 1.0)
            se_ps = mpsum.tile([1, local], F32, name="se_ps", tag="whT")
            nc.tensor.matmul(
                se_ps[:], ones_colf[0:nclass, :], ev[:], start=True, stop=True
            )
            lz = small.tile([1, local], F32, name="lz", tag="lz")
            nc.scalar.activation(lz[:], se_ps[:], AF.Ln)
            bz_ps = mpsum.tile([nclass, local], F32, name="bz_ps", tag="whT")
            nc.tensor.matmul(
                bz_ps[:], ones_row[:, 0:nclass], lz[:], start=True, stop=True
            )
            outT_sb = small.tile([nclass, local], F32, name="outT_sb", tag="outT")
            nc.vector.tensor_tensor(outT_sb[:], vt[:], bz_ps[:], ALU.subtract)
            nc.sync.dma_start(out_d[:, :], outT_sb[:])

    nc.compile()
    return nc


# ---------------------------------------------------------------------------
# Host side
# ---------------------------------------------------------------------------


def _prep_inputs(x, adj, W, a, W_out, a_out, ncores=NCORES):
    """Shard + lay out the numpy inputs for each core."""
    n, nfeat = x.shape
    nheads, _, nhid = W.shape
    nclass = W_out.shape[1]
    local = n // ncores

    x = np.asarray(x, np.float32)
    adjf = np.asarray(adj, np.float32)  # values {0,1}: mask == adj
    W = np.asarray(W, np.float32)
    a = np.asarray(a, np.float32)
    W_out = np.asarray(W_out, np.float32)
    a_out = np.asarray(a_out, np.float32)

    # W_aug per head, padded to 128 cols: [0:nhid]=W_h, 64=W_h@a1, 96=W_h@a2
    wa1 = np.einsum("hfd,hd->hf", W, a[:, :nhid])
    wa2 = np.einsum("hfd,hd->hf", W, a[:, nhid:])
    wall = np.zeros((nheads, nfeat, 128), np.float32)
    wall[:, :, :nhid] = W
    wall[:, :, 64] = wa1
    wall[:, :, 96] = wa2
    wall = wall.transpose(1, 0, 2).reshape(nfeat, nheads * 128)
    wall = np.ascontiguousarray(wall, dtype=NP_BF16)

    wouta = np.zeros((nfeat, 128), np.float32)
    wouta[:, :nclass] = W_out
    wouta[:, 64] = W_out @ a_out[:nclass]
    wouta[:, 96] = W_out @ a_out[nclass:]
    wout = np.concatenate([wouta, wouta.sum(axis=0, keepdims=True)], axis=0)
    wout = np.ascontiguousarray(wout, dtype=NP_BF16)
    onesrow = np.ones((1, n), dtype=NP_BF16)

    in_maps = []
    for c in range(ncores):
        rows = slice(c * local, (c + 1) * local)
        in_maps.append(
            {
                "xT": np.ascontiguousarray(x[rows].T, dtype=NP_BF16),
                "wall": wall,
                "wout": wout,
                "maskT": np.ascontiguousarray(adjf[rows].T, dtype=NP_BF16),
                "onesrow": onesrow,
            }
        )
    return in_maps


_NC_CACHE = {}


def kernel(x, adj, W, a, W_out, a_out):
    key = "full"
    if key not in _NC_CACHE:
        _NC_CACHE[key] = build_gat_nc()
    nc = _NC_CACHE[key]
    in_maps = _prep_inputs(x, adj, W, a, W_out, a_out)
    res = run_bass_kernel_spmd(nc, in_maps, core_ids=list(range(NCORES)))
    outs = [res.results[c]["outT"].T for c in range(NCORES)]
    return np.ascontiguousarray(np.concatenate(outs, axis=0), dtype=np.float32)


if __name__ == "__main__":
    nc = build_gat_nc()
    print("build + compile OK")


# revision 23
# speedup vs baseline: 4.8662x; 4.8662x over previous
"""GAT (2-layer, 8-head) Bass kernel for one TRN2 chip (8 NeuronCores).

Row-sharded: core c owns rows [c*L, (c+1)*L) of the N=4096 nodes.
All big on-chip tensors live in a TRANSPOSED layout [j (partitions), i (free)]
so that the softmax row-sum and att@Wh both run on the TensorEngine
(contraction over j = partition dim), and the attention logits
e[i,j] = leaky_relu(src[i]+dst[j]) are built by rank-2 outer-product matmuls.

Per layer: local Wh^T = W_aug^T x^T on PE (W_aug has W@a1, W@a2 folded in as
extra columns, so src/dst come out of the same matmul), AllGather of Wh + dst,
then per 4-j-tile chunk: PE outer-add -> ACT Lrelu -> ACT Exp -> DVE mask-mult
-> PE att@Wh(+rowsum via ones column). ELU / log_softmax tails on small
[H, 512] tensors. Output returned transposed [40, 512] per core; host
re-transposes.
"""

import sys

sys.path.insert(0, "/opt/trn_rl_repo")

import numpy as np
import ml_dtypes

from concourse import bass, bacc, mybir, tile
from concourse.bass_utils import run_bass_kernel_spmd

BF16 = mybir.dt.bfloat16
F32 = mybir.dt.float32
AF = mybir.ActivationFunctionType
ALU = mybir.AluOpType

NP_BF16 = ml_dtypes.bfloat16

# Problem constants (full size)
N = 4096
NCORES = 8
NFEAT = 512
NHID = 64
NCLASS = 40
NHEADS = 8
ALPHA = 0.2


def build_gat_nc(
    n=N,
    ncores=NCORES,
    nfeat=NFEAT,
    nhid=NHID,
    nclass=NCLASS,
    nheads=NHEADS,
    chunk_jt=4,
):
    """Build the SPMD Bass program (same program on every core)."""
    local = n // ncores  # rows per core
    njt = n // 128  # j tiles of 128 partitions
    jt_per_core = local // 128
    nft = nfeat // 128  # feature tiles (contraction dim of layer-1 matmul)
    assert local % 128 == 0 and n % 128 == 0 and nfeat % 128 == 0
    assert njt % chunk_jt == 0
    nchunks = njt // chunk_jt
    cw = chunk_jt * local  # chunk free-size in fp32 elements (psum)
    assert chunk_jt * 128 * 4 <= 8 * 2048  # chunk fits PSUM comfortably
    # W_aug padded to 128 cols so src/dst land on legal partition bases:
    # cols [0:nhid]=W, col 64=W@a1 (src), col 96=W@a2 (dst), zeros elsewhere
    w1c = 128
    w2c = 128

    nc = bacc.Bacc(
        "TRN2", target_bir_lowering=False, debug=False, num_devices=ncores
    )

    # ---------------- I/O ----------------
    xT_d = nc.dram_tensor("xT", [nfeat, local], BF16, kind="ExternalInput")
    wall_d = nc.dram_tensor(
        "wall", [nfeat, nheads * w1c], BF16, kind="ExternalInput"
    )
    # rows 0..nfeat-1 = [W_out | W_out@a1 | W_out@a2]; last row = column sums
    wout_d = nc.dram_tensor("wout", [nfeat + 1, w2c], BF16, kind="ExternalInput")
    maskT_d = nc.dram_tensor("maskT", [n, local], BF16, kind="ExternalInput")
    ones_d = nc.dram_tensor("onesrow", [1, n], BF16, kind="ExternalInput")
    ident_d = nc.dram_tensor("ident", [128, 128], BF16, kind="ExternalInput")
    out_d = nc.dram_tensor("outT", [nclass, local], F32, kind="ExternalOutput")

    with tile.TileContext(nc) as tc:
        with (
            tc.tile_pool(name="const", bufs=1) as const,
            tc.tile_pool(name="stage", bufs=2) as stage,
            tc.tile_pool(name="upool", bufs=2) as upool,
            tc.tile_pool(name="ppool", bufs=2) as ppool,
            tc.tile_pool(name="pmpool", bufs=2) as pmpool,
            tc.tile_pool(name="small", bufs=2) as small,
            tc.tile_pool(name="spsum", bufs=1, space="PSUM") as spsum,
            tc.tile_pool(name="attpsum", bufs=2, space="PSUM") as attpsum,
            tc.tile_pool(name="mpsum", bufs=2, space="PSUM") as mpsum,
            tc.tile_pool(name="dram", bufs=1, space="DRAM") as dram,
        ):
            # ---------------- static SBUF tensors ----------------
            xt = []
            wl = []
            for q in range(nft):
                xtq = const.tile([128, local], BF16, name=f"xt{q}", tag=f"xt{q}")
                nc.sync.dma_start(xtq[:], xT_d[q * 128 : (q + 1) * 128, :])
                xt.append(xtq)
                wlq = const.tile(
                    [128, nheads * w1c], BF16, name=f"wl{q}", tag=f"wl{q}"
                )
                nc.sync.dma_start(wlq[:], wall_d[q * 128 : (q + 1) * 128, :])
                wl.append(wlq)
            wo = []
            for q in range(nft):
                woq = const.tile([128, w2c], BF16, name=f"wo{q}", tag=f"wo{q}")
                nc.sync.dma_start(woq[:], wout_d[q * 128 : (q + 1) * 128, :])
                wo.append(woq)
            wcorr = const.tile([1, w2c], BF16, name="wcorr", tag="wcorr")
            nc.sync.dma_start(wcorr[:], wout_d[nfeat : nfeat + 1, :])

            # mask, [128, njt, local] bf16 (transposed adjacency, {0,1})
            mask_sb = const.tile([128, njt, local], BF16, name="mask_sb", tag="mask")
            mt_r = maskT_d.ap().rearrange("(jt p) i -> p jt i", p=128)
            ndma = 4
            per = njt // ndma
            for q in range(ndma):
                nc.sync.dma_start(
                    mask_sb[:, q * per : (q + 1) * per, :],
                    mt_r[:, q * per : (q + 1) * per, :],
                )

            ident = const.tile([128, 128], BF16, name="ident", tag="ident")
            nc.sync.dma_start(ident[:], ident_d[:, :])
            ones_row = const.tile([1, 128], F32, name="ones_row", tag="ones_row")
            nc.vector.memset(ones_row[:], 1.0)
            negones = const.tile([1, local], BF16, name="negones", tag="negones")
            nc.vector.memset(negones[:], -1.0)

            # outer-product operand tables. matmul operands must start at
            # partition 0/32/64, so pack 3 (dst,ones)/(ones,src) pairs per
            # 66-row tile at bases 0/32/64. Slot k (0..8): tile k//3, base
            # 32*(k%3). Slots 0..7 = heads, slot 8 = layer 2.
            do2t = []
            so2t = []
            for t in range(3):
                d = const.tile([66, n], BF16, name=f"do2_{t}", tag=f"do2_{t}")
                s = const.tile([66, local], BF16, name=f"so2_{t}", tag=f"so2_{t}")
                do2t.append(d)
                so2t.append(s)
            def pair(k):
                return do2t[k // 3], so2t[k // 3], 32 * (k % 3)
            for k in range(nheads + 1):
                d, s, b = pair(k)
                nc.sync.dma_start(d[b + 1 : b + 2, :], ones_d[:, :])
                nc.vector.memset(s[b : b + 1, :], 1.0)

            # gathered Wh in lhsT form + ones column: [128, njt, nhid+1]
            whaug = []
            for h in range(nheads):
                wa = const.tile(
                    [128, njt, nhid + 1], BF16, name=f"whaug{h}", tag=f"whaug{h}"
                )
                whaug.append(wa)
            whaug2 = const.tile([128, njt, 65], BF16, name="whaug2", tag="whaug2")

            # h~ = elu(layer1 out)+1, transposed: [nfeat=nheads*nhid, local]
            ht = []
            for q in range(nft):
                htq = const.tile([128, local], BF16, name=f"ht{q}", tag=f"ht{q}")
                ht.append(htq)

            # ---------------- DRAM bounce buffers ----------------
            # p-major payload: pay[p, h, jtl, dd] with dd = nhid cols + ones
            paycols = nheads * jt_per_core * (nhid + 1)
            wh_pay = dram.tile([128, paycols], BF16, name="wh_pay")
            sd_pay = dram.tile([nheads, local], BF16, name="sd_pay")
            wh_gath = dram.tile(
                [ncores * 128, paycols], BF16, name="wh_gath", addr_space="Shared"
            )
            sd_gath = dram.tile(
                [ncores * nheads, local], BF16, name="sd_gath", addr_space="Shared"
            )
            pay2cols = jt_per_core * 65
            wh2_pay = dram.tile([128, pay2cols], BF16, name="wh2_pay")
            sd2_pay = dram.tile([1, local], BF16, name="sd2_pay")
            wh2_gath = dram.tile(
                [ncores * 128, pay2cols], BF16, name="wh2_gath", addr_space="Shared"
            )
            sd2_gath = dram.tile(
                [ncores, local], BF16, name="sd2_gath", addr_space="Shared"
            )

            # ---------------- phase 1: local Wh^T per head ----------------
            # pay_sb[p, h, jtl, dd]: Wh_local[jtl*128+p, d] + ones col dd=nhid
            pay_sb = const.tile(
                [128, nheads, jt_per_core, nhid + 1], BF16, name="pay_sb", tag="pay_sb"
            )
            nc.vector.memset(pay_sb[:, :, :, nhid : nhid + 1], 1.0)
            for h in range(nheads):
                whT_ps = mpsum.tile([w1c, local], F32, name=f"whT{h}", tag="whT")
                for q in range(nft):
                    nc.tensor.matmul(
                        whT_ps[:],
                        wl[q][:, h * w1c : (h + 1) * w1c],
                        xt[q][:],
                        start=(q == 0),
                        stop=(q == nft - 1),
                    )
                whT_sb = stage.tile([nhid, local], BF16, name="whT_sb", tag="whT_sb")
                nc.scalar.copy(whT_sb[:], whT_ps[0:nhid, :])
                # transpose [nhid, 128] blocks -> [128, nhid] into payload
                for q in range(jt_per_core):
                    tp_ps = mpsum.tile([128, nhid], BF16, name="tp_ps", tag="whT")
                    nc.tensor.matmul(
                        tp_ps[:],
                        whT_sb[:, q * 128 : (q + 1) * 128],
                        ident[0:nhid, 0:nhid],
                        is_transpose=True,
                    )
                    nc.vector.tensor_copy(pay_sb[:, h, q, 0:nhid], tp_ps[:])
                # src row (64) -> so2 (via SBUF-to-SBUF DMA; odd partition
                # writes are not legal for compute engines), dst row (96) ->
                # sd payload
                sh, ssb, sb_ = pair(h)
                src_sb = stage.tile([1, local], BF16, name="src_sb", tag="src_sb")
                nc.scalar.copy(src_sb[:], whT_ps[64:65, :])
                nc.sync.dma_start(ssb[sb_ + 1 : sb_ + 2, :], src_sb[:])
                dst_sb = stage.tile([1, local], BF16, name="dst_sb", tag="dst_sb")
                nc.scalar.copy(dst_sb[:], whT_ps[96:97, :])
                nc.sync.dma_start(sd_pay[h : h + 1, :], dst_sb[:])
            nc.sync.dma_start(
                wh_pay[:].rearrange("p (h jtl dd) -> p h jtl dd", h=nheads, jtl=jt_per_core),
                pay_sb[:],
            )

            # ---------------- phase 2: AllGather ----------------
            grp = [list(range(ncores))]
            nc.gpsimd.collective_compute(
                "AllGather",
                ALU.bypass,
                replica_groups=grp,
                ins=[wh_pay[:].opt()],
                outs=[wh_gath[:].opt()],
            )
            nc.gpsimd.collective_compute(
                "AllGather",
                ALU.bypass,
                replica_groups=grp,
                ins=[sd_pay[:].opt()],
                outs=[sd_gath[:].opt()],
            )

            # ---------------- phase 3: build whaug, do2 ----------------
            # wh_gath: per core c a [128, h, jtl, dd] p-major block; the
            # whaug slice [:, c*jpc:(c+1)*jpc, :] is contiguous per partition
            wg = wh_gath[:].rearrange(
                "(c p) (h jtl dd) -> c p h jtl dd",
                c=ncores,
                h=nheads,
                jtl=jt_per_core,
            )
            eng = [nc.sync, nc.gpsimd, nc.scalar]
            for h in range(nheads):
                for c in range(ncores):
                    eng[(h * ncores + c) % len(eng)].dma_start(
                        whaug[h][:, c * jt_per_core : (c + 1) * jt_per_core, :],
                        wg[c, :, h],
                    )
            # sd_gath row = c*H + h -> do2 slot-row for head h, [1, ncores, local]
            sg = sd_gath[:].rearrange("(c h) i -> h c i", h=nheads)
            for h in range(nheads):
                d, _, b = pair(h)
                d_r = d[b : b + 1, :].rearrange("p (c i) -> p c i", c=ncores)
                nc.sync.dma_start(d_r, sg[h : h + 1])

            # ---------------- phase 4: layer-1 attention main loop ----------
            def att_layer(nh, whaug_h, slot):
                """One head's attention: att psum tile [65, local], rowsum row 64."""
                do2_t, so2_t, b = pair(slot)
                att_ps = attpsum.tile([65, local], F32, name="att_ps", tag="att")
                for ch in range(nchunks):
                    s_ps = spsum.tile([128, cw], F32, name="s_ps", tag="s_ps")
                    for q in range(chunk_jt):
                        jt = ch * chunk_jt + q
                        nc.tensor.matmul(
                            s_ps[:, q * local : (q + 1) * local],
                            do2_t[b : b + 2, jt * 128 : (jt + 1) * 128],
                            so2_t[b : b + 2, :],
                            start=True,
                            stop=True,
                        )
                    # exp(lrelu(s)) = max(exp(s), exp(alpha*s)) -- same Exp
                    # table for both passes (no ACT table reloads)
                    e1 = upool.tile([128, cw], BF16, name="e1", tag="u")
                    nc.scalar.activation(e1[:], s_ps[:], AF.Exp)
                    e2 = ppool.tile([128, cw], BF16, name="e2", tag="p")
                    nc.scalar.activation(e2[:], s_ps[:], AF.Exp, scale=ALPHA)
                    p = pmpool.tile([128, cw], BF16, name="p", tag="pm")
                    nc.vector.tensor_tensor(p[:], e1[:], e2[:], ALU.max)
                    pm = pmpool.tile([128, cw], BF16, name="pm", tag="pm")
                    pv = p[:].rearrange("p (q i) -> p q i", q=chunk_jt)
                    pmv = pm[:].rearrange("p (q i) -> p q i", q=chunk_jt)
                    nc.vector.tensor_tensor(
                        pmv,
                        pv,
                        mask_sb[:, ch * chunk_jt : (ch + 1) * chunk_jt, :],
                        ALU.mult,
                    )
                    for q in range(chunk_jt):
                        jt = ch * chunk_jt + q
                        nc.tensor.matmul(
                            att_ps[:],
                            whaug_h[:, jt, :],
                            pm[:, q * local : (q + 1) * local],
                            start=(jt == 0),
                            stop=(jt == njt - 1),
                        )
                return att_ps

            def elu_plus1(z_sb, out_view):
                """out = elu(z)+1 = (z>=0 ? 1+z : exp(z)), elementwise."""
                pp = z_sb.shape[0]
                m1 = small.tile([pp, local], F32, name="m1", tag="elu_m1")
                nc.vector.tensor_scalar(m1[:], z_sb[:], 0.0, None, ALU.min)
                e1 = small.tile([pp, local], F32, name="e1", tag="elu_e1")
                nc.scalar.activation(e1[:], m1[:], AF.Exp)
                r1 = small.tile([pp, local], F32, name="r1", tag="elu_r1")
                nc.vector.tensor_scalar(r1[:], z_sb[:], 0.0, None, ALU.max)
                nc.vector.tensor_tensor(out_view, e1[:], r1[:], ALU.add)

            for h in range(nheads):
                att_ps = att_layer(nhid, whaug[h], h)
                # normalize + elu + 1 -> ht
                rc = small.tile([1, local], F32, name="rc", tag="rc")
                nc.vector.reciprocal(rc[:], att_ps[64:65, :])
                bc_ps = mpsum.tile([nhid, local], F32, name="bc_ps", tag="whT")
                nc.tensor.matmul(
                    bc_ps[:], ones_row[:, 0:nhid], rc[:], start=True, stop=True
                )
                bc_sb = small.tile([nhid, local], F32, name="bc_sb", tag="bc_sb")
                nc.scalar.copy(bc_sb[:], bc_ps[:])
                z_sb = small.tile([nhid, local], F32, name="z_sb", tag="z_sb")
                nc.vector.tensor_tensor(z_sb[:], att_ps[0:nhid, :], bc_sb[:], ALU.mult)
                q, r = divmod(h * nhid, 128)
                elu_plus1(z_sb, ht[q][r : r + nhid, :])

            # ---------------- phase 5: layer 2 ----------------
            wh2T_ps = mpsum.tile([w2c, local], F32, name="wh2T", tag="whT")
            for q in range(nft):
                nc.tensor.matmul(
                    wh2T_ps[:], wo[q][:], ht[q][:], start=(q == 0), stop=False
                )
            # subtract column-sums (the h~ = h+1 correction)
            nc.tensor.matmul(
                wh2T_ps[:], wcorr[:], negones[:], start=False, stop=True
            )
            wh2_sb = stage.tile([nclass, local], BF16, name="wh2_sb", tag="wh2_sb")
            nc.scalar.copy(wh2_sb[:], wh2T_ps[0:nclass, :])
            # pay2_sb[p, jtl, dd]: cols 0:nclass Wh2, nclass:64 zero, 64 ones
            pay2_sb = const.tile(
                [128, jt_per_core, 65], BF16, name="pay2_sb", tag="pay2_sb"
            )
            nc.vector.memset(pay2_sb[:, :, nclass:64], 0.0)
            nc.vector.memset(pay2_sb[:, :, 64:65], 1.0)
            for q in range(jt_per_core):
                tp2_ps = mpsum.tile([128, nclass], BF16, name="tp2_ps", tag="whT")
                nc.tensor.matmul(
                    tp2_ps[:],
                    wh2_sb[:, q * 128 : (q + 1) * 128],
                    ident[0:nclass, 0:nclass],
                    is_transpose=True,
                )
                nc.vector.tensor_copy(pay2_sb[:, q, 0:nclass], tp2_ps[:])
            nc.sync.dma_start(
                wh2_pay[:].rearrange("p (jtl dd) -> p jtl dd", jtl=jt_per_core),
                pay2_sb[:],
            )
            d2t, s2t, b2 = pair(nheads)
            src2_sb = stage.tile([1, local], BF16, name="src2_sb", tag="src_sb")
            nc.scalar.copy(src2_sb[:], wh2T_ps[64:65, :])
            nc.sync.dma_start(s2t[b2 + 1 : b2 + 2, :], src2_sb[:])
            dst2_sb = stage.tile([1, local], BF16, name="dst2_sb", tag="dst_sb")
            nc.scalar.copy(dst2_sb[:], wh2T_ps[96:97, :])
            nc.sync.dma_start(sd2_pay[:], dst2_sb[:])

            nc.gpsimd.collective_compute(
                "AllGather",
                ALU.bypass,
                replica_groups=grp,
                ins=[wh2_pay[:].opt()],
                outs=[wh2_gath[:].opt()],
            )
            nc.gpsimd.collective_compute(
                "AllGather",
                ALU.bypass,
                replica_groups=grp,
                ins=[sd2_pay[:].opt()],
                outs=[sd2_gath[:].opt()],
            )

            wg2 = wh2_gath[:].rearrange(
                "(c p) (jtl dd) -> c p jtl dd", c=ncores, jtl=jt_per_core
            )
            for c in range(ncores):
                eng[c % len(eng)].dma_start(
                    whaug2[:, c * jt_per_core : (c + 1) * jt_per_core, :],
                    wg2[c],
                )
            do22_r = d2t[b2 : b2 + 1, :].rearrange("p (c i) -> p c i", c=ncores)
            nc.sync.dma_start(
                do22_r, sd2_gath[:].rearrange("(o c) i -> o c i", o=1)
            )

            att2_ps = att_layer(nclass, whaug2, nheads)

            # ---------------- phase 6: final elu + log_softmax ------------
            rc2 = small.tile([1, local], F32, name="rc2", tag="rc")
            nc.vector.reciprocal(rc2[:], att2_ps[64:65, :])
            bc2_ps = mpsum.tile([nclass, local], F32, name="bc2_ps", tag="whT")
            nc.tensor.matmul(
                bc2_ps[:], ones_row[:, 0:nclass], rc2[:], start=True, stop=True
            )
            bc2_sb = small.tile([nclass, local], F32, name="bc2_sb", tag="bc_sb")
            nc.scalar.copy(bc2_sb[:], bc2_ps[:])
            v_sb = small.tile([nclass, local], F32, name="v_sb", tag="z_sb")
            nc.vector.tensor_tensor(v_sb[:], att2_ps[0:nclass, :], bc2_sb[:], ALU.mult)
            vt = small.tile([nclass, local], F32, name="vt", tag="vt")
            elu_plus1(v_sb, vt[:])
            # log_softmax(v-1) = vt - ln(sum exp(vt))  (the -1 cancels)
            ev = small.tile([nclass, local], F32, name="ev", tag="ev")
            nc.scalar.activation(ev[:], vt[:], AF.Exp)
            ones_colf = const.tile([128, 1], F32, name="ones_colf", tag="ones_colf")
            nc.vector.memset(ones_colf[:], 1.0)
            se_ps = mpsum.tile([1, local], F32, name="se_ps", tag="whT")
            nc.tensor.matmul(
                se_ps[:], ones_colf[0:nclass, :], ev[:], start=True, stop=True
            )
            lz = small.tile([1, local], F32, name="lz", tag="lz")
            nc.scalar.activation(lz[:], se_ps[:], AF.Ln)
            bz_ps = mpsum.tile([nclass, local], F32, name="bz_ps", tag="whT")
            nc.tensor.matmul(
                bz_ps[:], ones_row[:, 0:nclass], lz[:], start=True, stop=True
            )
            outT_sb = small.tile([nclass, local], F32, name="outT_sb", tag="outT")
            nc.vector.tensor_tensor(outT_sb[:], vt[:], bz_ps[:], ALU.subtract)
            nc.sync.dma_start(out_d[:, :], outT_sb[:])

    nc.compile()
    return nc


# ---------------------------------------------------------------------------
# Host side
# ---------------------------------------------------------------------------


def _prep_inputs(x, adj, W, a, W_out, a_out, ncores=NCORES):
    """Shard + lay out the numpy inputs for each core."""
    n, nfeat = x.shape
    nheads, _, nhid = W.shape
    nclass = W_out.shape[1]
    local = n // ncores

    x = np.asarray(x, np.float32)
    adjf = np.asarray(adj, np.float32)  # values {0,1}: mask == adj
    W = np.asarray(W, np.float32)
    a = np.asarray(a, np.float32)
    W_out = np.asarray(W_out, np.float32)
    a_out = np.asarray(a_out, np.float32)

    # W_aug per head, padded to 128 cols: [0:nhid]=W_h, 64=W_h@a1, 96=W_h@a2
    wa1 = np.einsum("hfd,hd->hf", W, a[:, :nhid])
    wa2 = np.einsum("hfd,hd->hf", W, a[:, nhid:])
    wall = np.zeros((nheads, nfeat, 128), np.float32)
    wall[:, :, :nhid] = W
    wall[:, :, 64] = wa1
    wall[:, :, 96] = wa2
    wall = wall.transpose(1, 0, 2).reshape(nfeat, nheads * 128)
    wall = np.ascontiguousarray(wall, dtype=NP_BF16)

    wouta = np.zeros((nfeat, 128), np.float32)
    wouta[:, :nclass] = W_out
    wouta[:, 64] = W_out @ a_out[:nclass]
    wouta[:, 96] = W_out @ a_out[nclass:]
    wout = np.concatenate([wouta, wouta.sum(axis=0, keepdims=True)], axis=0)
    wout = np.ascontiguousarray(wout, dtype=NP_BF16)
    onesrow = np.ones((1, n), dtype=NP_BF16)
    ident = np.eye(128, dtype=NP_BF16)

    in_maps = []
    for c in range(ncores):
        rows = slice(c * local, (c + 1) * local)
        in_maps.append(
            {
                "xT": np.ascontiguousarray(x[rows].T, dtype=NP_BF16),
                "wall": wall,
                "wout": wout,
                "maskT": np.ascontiguousarray(adjf[rows].T, dtype=NP_BF16),
                "onesrow": onesrow,
                "ident": ident,
            }
        )
    return in_maps


_NC_CACHE = {}


def kernel(x, adj, W, a, W_out, a_out):
    key = "full"
    if key not in _NC_CACHE:
        _NC_CACHE[key] = build_gat_nc()
    nc = _NC_CACHE[key]
    in_maps = _prep_inputs(x, adj, W, a, W_out, a_out)
    res = run_bass_kernel_spmd(nc, in_maps, core_ids=list(range(NCORES)))
    outs = [res.results[c]["outT"].T for c in range(NCORES)]
    return np.ascontiguousarray(np.concatenate(outs, axis=0), dtype=np.float32)


if __name__ == "__main__":
    nc = build_gat_nc()
    print("build + compile OK")
